# revision 21
# baseline (speedup 1.0000x reference)
"""Trainium2 Bass kernel for nn_AudioModel segment_reduce.

Reference computation (per batch row b):
  - frames t < audio_lengths[b] are valid
  - consecutive runs of equal phoneme_ids form segments
  - feat[b] = mean over segments of (mean over frames in segment of h[b,t,:])
  - logit[b] = feat[b] @ W.T + bias

Algebraic collapse: feat[b] = sum_t w[b,t] * h[b,t,:] with
  w[b,t] = valid[b,t] / (run_len(b, run_of(t)) * n_runs[b])
so  logit[b] = sum_t w[b,t] * (h[b,t,:] . W) + bias.

The per-frame weights w depend only on the tiny phoneme_ids/audio_lengths
tensors and are computed on host. The 588MB hidden_states tensor is streamed
through the device exactly once (memory-bound): per batch row, the T
contraction runs on DVE as a chain of fused scalar_tensor_tensor ops
(acc += w_chunk * h_chunk, per-partition scalar = per-frame weight), the
remaining chunks contract on the PE as fp32 matmuls accumulating in PSUM
(the DVE accumulator folds into the same PSUM group via a ones-matmul), and
one fused DVE op applies the classifier dot. Full fp32 throughout. Pure data
parallel: 16 batch rows per NeuronCore; each row's loads are split into
~1.1MB 3-chunk pieces across the two HWDGE rings (sync/scalar feed DVE/PE
respectively, alternating per row) with the ragged 88-frame tail on SWDGE,
so the HBM stream stays dense end-to-end and both pipeline edges are short.
"""

import numpy as np

B, T, C = 128, 1496, 768
NCORES = 8
RPC = B // NCORES          # batch rows per core
NCHUNK = 12                # t-chunks of 128 frames
NTAIL = T - 11 * 128       # 88 frames in the last chunk
DVE_CHUNKS = 6             # chunks 0..5 + tail on DVE, 6..10 on PE

_CACHE = {}


def _frame_weights(phoneme_ids, audio_lengths):
    """w[b,t] = valid / (run_length(run containing t) * n_runs[b]); 0 if invalid."""
    pid = np.asarray(phoneme_ids)
    L = np.asarray(audio_lengths).astype(np.int64)
    t = np.arange(T)
    valid = t[None, :] < L[:, None]                               # (B, T)
    change = pid[:, 1:] != pid[:, :-1]
    boundary = np.concatenate([np.ones((B, 1), bool), change], axis=1) & valid
    seg = np.cumsum(boundary, axis=1) - 1
    np.maximum(seg, 0, out=seg)                                   # (B, T)
    gid = (seg + np.arange(B, dtype=np.int64)[:, None] * T).ravel()
    cnt = np.bincount(gid, weights=valid.ravel().astype(np.float64), minlength=B * T)
    cnt_t = cnt[gid].reshape(B, T)                                # run length per frame
    n_runs = boundary.sum(axis=1).astype(np.float64)              # (B,)
    w = np.where(valid, 1.0 / (np.maximum(cnt_t, 1.0) * n_runs[:, None]), 0.0)
    return w.astype(np.float32)


def _weight_cols(w):
    """(B, 128, NCHUNK): [:, p, ci] = weight of frame ci*128+p (zeros pad the tail)."""
    wc = np.zeros((B, 128, NCHUNK), dtype=np.float32)
    wc[:, :, :11] = w[:, : 11 * 128].reshape(B, 11, 128).transpose(0, 2, 1)
    wc[:, :NTAIL, 11] = w[:, 11 * 128 :]
    return wc


def _build_program():
    import concourse.bacc as bacc
    import concourse.tile as tile
    from concourse import mybir

    f32 = mybir.dt.float32

    nc = bacc.Bacc("TRN2", target_bir_lowering=False, debug=False)
    h = nc.dram_tensor("h", [RPC, T, C], f32, kind="ExternalInput").ap()
    wt = nc.dram_tensor("wt", [128, RPC * NCHUNK], f32, kind="ExternalInput").ap()
    wv = nc.dram_tensor("wv", [1, C], f32, kind="ExternalInput").ap()
    out = nc.dram_tensor("out", [RPC, 1], f32, kind="ExternalOutput").ap()

    with tile.TileContext(nc) as tc:
        with (
            tc.tile_pool(name="hp", bufs=4) as hp,
            tc.tile_pool(name="const", bufs=1) as cp,
            tc.tile_pool(name="ap_", bufs=4) as apool,
            tc.tile_pool(name="ps", bufs=4, space="PSUM") as pp,
            tc.tile_pool(name="fin", bufs=3) as fp,
        ):
            wtile = cp.tile([128, RPC * NCHUNK], f32)
            nc.gpsimd.dma_start(wtile[:], wt)
            wvt = cp.tile([1, C], f32)
            nc.gpsimd.dma_start(wvt[:], wv)
            logits = cp.tile([1, RPC], f32)
            ones = cp.tile([128, 1], f32)
            nc.vector.memset(ones[:], 1.0)

            NA = DVE_CHUNKS            # chunks 0..NA-1 -> DVE (tile A, ring 0)
            NB = 11 - NA               # chunks NA..10  -> PE  (tile B, ring 1)
            for r in range(RPC):
                # Per-row loads are split so each engine's data arrives on its
                # own ring and compute starts after a half-row, not a full row:
                #   A: chunks 0..NA-1  -> DVE accumulator chain
                #   B: chunks NA..10   -> PE fp32 matmul group
                #   T: ragged 88-frame tail (SWDGE) -> one more DVE chain op
                ra, rb = (nc.sync, nc.scalar) if r % 2 == 0 else (nc.scalar, nc.sync)
                hA = hp.tile([128, NA * C], f32, tag="hA")
                hA3 = hA.rearrange("p (ci c) -> p ci c", c=C)
                for lo, hi in ((0, 3), (3, NA)):
                    ra.dma_start(
                        hA3[:, lo:hi, :],
                        h[r, lo * 128 : hi * 128, :].rearrange(
                            "(ci p) c -> p ci c", p=128
                        ),
                    )
                hB = hp.tile([128, NB * C], f32, tag="hB")
                hB3 = hB.rearrange("p (ci c) -> p ci c", c=C)
                for lo, hi in ((0, 3), (3, NB)):
                    rb.dma_start(
                        hB3[:, lo:hi, :],
                        h[r, (NA + lo) * 128 : (NA + hi) * 128, :].rearrange(
                            "(ci p) c -> p ci c", p=128
                        ),
                    )
                hT = hp.tile([128, C], f32, tag="hT")
                nc.gpsimd.dma_start(hT[:NTAIL, :], h[r, 11 * 128 :, :])

                col = lambda ci: wtile[:, r * NCHUNK + ci : r * NCHUNK + ci + 1]
                acc = apool.tile([128, C], f32, tag="acc")
                nc.vector.tensor_scalar_mul(acc[:], hA[:, :C], col(0))
                for ci in range(1, NA):
                    nc.vector.scalar_tensor_tensor(
                        out=acc[:],
                        in0=hA[:, ci * C : (ci + 1) * C],
                        scalar=col(ci),
                        in1=acc[:],
                        op0=mybir.AluOpType.mult,
                        op1=mybir.AluOpType.add,
                    )
                nc.vector.scalar_tensor_tensor(
                    out=acc[:NTAIL],
                    in0=hT[:NTAIL, :],
                    scalar=col(11)[:NTAIL],
                    in1=acc[:NTAIL],
                    op0=mybir.AluOpType.mult,
                    op1=mybir.AluOpType.add,
                )

                p = pp.tile([1, C], f32, tag="p")
                for k, ci in enumerate(range(NA, 11)):
                    lw = col(ci)
                    nc.tensor.matmul(
                        p[:, :512], lw, hB3[:, k, :512], start=(k == 0), stop=False
                    )
                    nc.tensor.matmul(
                        p[:, 512:], lw, hB3[:, k, 512:], start=(k == 0), stop=False
                    )
                nc.tensor.matmul(p[:, :512], ones[:], acc[:, :512], start=False, stop=True)
                nc.tensor.matmul(p[:, 512:], ones[:], acc[:, 512:], start=False, stop=True)
                sc = fp.tile([1, C], f32, tag="sc")
                nc.vector.scalar_tensor_tensor(
                    out=sc[:],
                    in0=p[:],
                    scalar=1.0,
                    in1=wvt[:],
                    op0=mybir.AluOpType.mult,
                    op1=mybir.AluOpType.mult,
                    accum_out=logits[:, r : r + 1],
                )

            nc.sync.dma_start(out.rearrange("r o -> o r"), logits[:])

    nc.compile()
    return nc


def _get_program():
    if "nc" not in _CACHE:
        _CACHE["nc"] = _build_program()
    return _CACHE["nc"]


def _run(inputs, trace=False):
    from concourse.bass_utils import run_bass_kernel_spmd

    hidden = np.ascontiguousarray(np.asarray(inputs["hidden_states"], dtype=np.float32))
    W = np.ascontiguousarray(np.asarray(inputs["W"], dtype=np.float32))
    bias = np.asarray(inputs["b"], dtype=np.float32)
    w = _frame_weights(inputs["phoneme_ids"], inputs["audio_lengths"])
    wc = _weight_cols(w)                                          # (B, 128, NCHUNK)

    in_maps = []
    for i in range(NCORES):
        r0 = i * RPC
        wt_core = np.ascontiguousarray(
            wc[r0 : r0 + RPC].transpose(1, 0, 2).reshape(128, RPC * NCHUNK)
        )
        in_maps.append(
            {
                "h": hidden[r0 : r0 + RPC],
                "wt": wt_core,
                "wv": W.reshape(1, C),
            }
        )

    nc = _get_program()
    res = run_bass_kernel_spmd(nc, in_maps, list(range(NCORES)), trace=trace)
    logit = np.concatenate([res.results[i]["out"] for i in range(NCORES)], axis=0)
    logit = logit + bias[None, :]
    return logit.astype(np.float32), res


def kernel(**inputs):
    return _run(inputs, trace=False)[0]


# revision 23
# speedup vs baseline: 1.0080x; 1.0080x over previous
"""Trainium2 Bass kernel for nn_AudioModel segment_reduce.

Reference computation (per batch row b):
  - frames t < audio_lengths[b] are valid
  - consecutive runs of equal phoneme_ids form segments
  - feat[b] = mean over segments of (mean over frames in segment of h[b,t,:])
  - logit[b] = feat[b] @ W.T + bias

Algebraic collapse: feat[b] = sum_t w[b,t] * h[b,t,:] with
  w[b,t] = valid[b,t] / (run_len(b, run_of(t)) * n_runs[b])
so  logit[b] = sum_t w[b,t] * (h[b,t,:] . W) + bias.

The per-frame weights w depend only on the tiny phoneme_ids/audio_lengths
tensors and are computed on host. The 588MB hidden_states tensor is streamed
through the device exactly once (memory-bound): per batch row, the T
contraction runs on DVE as a chain of fused scalar_tensor_tensor ops
(acc += w_chunk * h_chunk, per-partition scalar = per-frame weight), the
remaining chunks contract on the PE as fp32 matmuls accumulating in PSUM
(the DVE accumulator folds into the same PSUM group via a ones-matmul), and
one fused DVE op applies the classifier dot. Full fp32 throughout. Pure data
parallel: 16 batch rows per NeuronCore; each row's loads are split into
~1.1MB 3-chunk pieces across the two HWDGE rings (sync/scalar feed DVE/PE
respectively, alternating per row) with the ragged 88-frame tail on SWDGE,
so the HBM stream stays dense end-to-end and both pipeline edges are short.
"""

import numpy as np

B, T, C = 128, 1496, 768
NCORES = 8
RPC = B // NCORES          # batch rows per core
NCHUNK = 12                # t-chunks of 128 frames
NTAIL = T - 11 * 128       # 88 frames in the last chunk
DVE_CHUNKS = 6             # chunks 0..5 + tail on DVE, 6..10 on PE

_CACHE = {}


def _frame_weights(phoneme_ids, audio_lengths):
    """w[b,t] = valid / (run_length(run containing t) * n_runs[b]); 0 if invalid."""
    pid = np.asarray(phoneme_ids)
    L = np.asarray(audio_lengths).astype(np.int64)
    t = np.arange(T)
    valid = t[None, :] < L[:, None]                               # (B, T)
    change = pid[:, 1:] != pid[:, :-1]
    boundary = np.concatenate([np.ones((B, 1), bool), change], axis=1) & valid
    seg = np.cumsum(boundary, axis=1) - 1
    np.maximum(seg, 0, out=seg)                                   # (B, T)
    gid = (seg + np.arange(B, dtype=np.int64)[:, None] * T).ravel()
    cnt = np.bincount(gid, weights=valid.ravel().astype(np.float64), minlength=B * T)
    cnt_t = cnt[gid].reshape(B, T)                                # run length per frame
    n_runs = boundary.sum(axis=1).astype(np.float64)              # (B,)
    w = np.where(valid, 1.0 / (np.maximum(cnt_t, 1.0) * n_runs[:, None]), 0.0)
    return w.astype(np.float32)


def _weight_cols(w):
    """(B, 128, NCHUNK): [:, p, ci] = weight of frame ci*128+p (zeros pad the tail)."""
    wc = np.zeros((B, 128, NCHUNK), dtype=np.float32)
    wc[:, :, :11] = w[:, : 11 * 128].reshape(B, 11, 128).transpose(0, 2, 1)
    wc[:, :NTAIL, 11] = w[:, 11 * 128 :]
    return wc


def _build_program():
    import concourse.bacc as bacc
    import concourse.tile as tile
    from concourse import mybir

    f32 = mybir.dt.float32

    nc = bacc.Bacc("TRN2", target_bir_lowering=False, debug=False)
    h = nc.dram_tensor("h", [RPC, T, C], f32, kind="ExternalInput").ap()
    wt = nc.dram_tensor("wt", [128, RPC * NCHUNK], f32, kind="ExternalInput").ap()
    wv = nc.dram_tensor("wv", [1, C], f32, kind="ExternalInput").ap()
    out = nc.dram_tensor("out", [RPC, 1], f32, kind="ExternalOutput").ap()

    with tile.TileContext(nc) as tc:
        with (
            tc.tile_pool(name="hp", bufs=4) as hp,
            tc.tile_pool(name="const", bufs=1) as cp,
            tc.tile_pool(name="ap_", bufs=4) as apool,
            tc.tile_pool(name="ps", bufs=4, space="PSUM") as pp,
            tc.tile_pool(name="fin", bufs=3) as fp,
        ):
            wtile = cp.tile([128, RPC * NCHUNK], f32)
            nc.gpsimd.dma_start(wtile[:], wt)
            wvt = cp.tile([1, C], f32)
            nc.gpsimd.dma_start(wvt[:], wv)
            logits = cp.tile([1, RPC], f32)
            ones = cp.tile([128, 1], f32)
            nc.vector.memset(ones[:], 1.0)

            NA = DVE_CHUNKS            # chunks 0..NA-1 -> DVE (tile A, ring 0)
            NB = 11 - NA               # chunks NA..10  -> PE  (tile B, ring 1)
            for r in range(RPC):
                # Per-row loads are split so each engine's data arrives on its
                # own ring and compute starts after a half-row, not a full row:
                #   A: chunks 0..NA-1  -> DVE accumulator chain
                #   B: chunks NA..10   -> PE fp32 matmul group
                #   T: ragged 88-frame tail (SWDGE) -> one more DVE chain op
                ra, rb = (nc.sync, nc.scalar) if r % 2 == 0 else (nc.scalar, nc.sync)
                hA = hp.tile([128, NA * C], f32, tag="hA")
                hA3 = hA.rearrange("p (ci c) -> p ci c", c=C)
                for lo, hi in ((0, 3), (3, NA)):
                    ra.dma_start(
                        hA3[:, lo:hi, :],
                        h[r, lo * 128 : hi * 128, :].rearrange(
                            "(ci p) c -> p ci c", p=128
                        ),
                    )
                hB = hp.tile([128, NB * C], f32, tag="hB")
                hB3 = hB.rearrange("p (ci c) -> p ci c", c=C)
                for lo, hi in ((0, 3), (3, NB)):
                    rb.dma_start(
                        hB3[:, lo:hi, :],
                        h[r, (NA + lo) * 128 : (NA + hi) * 128, :].rearrange(
                            "(ci p) c -> p ci c", p=128
                        ),
                    )
                hT = hp.tile([128, C], f32, tag="hT")
                nc.gpsimd.dma_start(hT[:NTAIL, :], h[r, 11 * 128 :, :])

                col = lambda ci: wtile[:, r * NCHUNK + ci : r * NCHUNK + ci + 1]
                acc = apool.tile([128, C], f32, tag="acc")
                nc.vector.tensor_scalar_mul(acc[:], hA[:, :C], col(0))
                for ci in range(1, NA):
                    nc.vector.scalar_tensor_tensor(
                        out=acc[:],
                        in0=hA[:, ci * C : (ci + 1) * C],
                        scalar=col(ci),
                        in1=acc[:],
                        op0=mybir.AluOpType.mult,
                        op1=mybir.AluOpType.add,
                    )
                nc.vector.scalar_tensor_tensor(
                    out=acc[:NTAIL],
                    in0=hT[:NTAIL, :],
                    scalar=col(11)[:NTAIL],
                    in1=acc[:NTAIL],
                    op0=mybir.AluOpType.mult,
                    op1=mybir.AluOpType.add,
                )

                p = pp.tile([1, C], f32, tag="p")
                for k, ci in enumerate(range(NA, 11)):
                    lw = col(ci)
                    nc.tensor.matmul(
                        p[:, :512], lw, hB3[:, k, :512], start=(k == 0), stop=False
                    )
                    nc.tensor.matmul(
                        p[:, 512:], lw, hB3[:, k, 512:], start=(k == 0), stop=False
                    )
                nc.tensor.matmul(p[:, :512], ones[:], acc[:, :512], start=False, stop=True)
                nc.tensor.matmul(p[:, 512:], ones[:], acc[:, 512:], start=False, stop=True)
                sc = fp.tile([1, C], f32, tag="sc")
                nc.vector.scalar_tensor_tensor(
                    out=sc[:],
                    in0=p[:],
                    scalar=1.0,
                    in1=wvt[:],
                    op0=mybir.AluOpType.mult,
                    op1=mybir.AluOpType.mult,
                    accum_out=logits[:, r : r + 1],
                )

            nc.sync.dma_start(out.rearrange("r o -> o r"), logits[:])

    nc.compile()
    return nc


def _get_program():
    if "nc" not in _CACHE:
        _CACHE["nc"] = _build_program()
    return _CACHE["nc"]


def _run(inputs, trace=False):
    from concourse.bass_utils import run_bass_kernel_spmd

    hidden = np.ascontiguousarray(np.asarray(inputs["hidden_states"], dtype=np.float32))
    W = np.ascontiguousarray(np.asarray(inputs["W"], dtype=np.float32))
    bias = np.asarray(inputs["b"], dtype=np.float32)
    w = _frame_weights(inputs["phoneme_ids"], inputs["audio_lengths"])
    wc = _weight_cols(w)                                          # (B, 128, NCHUNK)

    in_maps = []
    for i in range(NCORES):
        r0 = i * RPC
        wt_core = np.ascontiguousarray(
            wc[r0 : r0 + RPC].transpose(1, 0, 2).reshape(128, RPC * NCHUNK)
        )
        in_maps.append(
            {
                "h": hidden[r0 : r0 + RPC],
                "wt": wt_core,
                "wv": W.reshape(1, C),
            }
        )

    nc = _get_program()
    res = run_bass_kernel_spmd(nc, in_maps, list(range(NCORES)), trace=trace)
    logit = np.concatenate([res.results[i]["out"] for i in range(NCORES)], axis=0)
    logit = logit + bias[None, :]
    return logit.astype(np.float32), res


def kernel(**inputs):
    return _run(inputs, trace=False)[0]


# revision 24
# speedup vs baseline: 1.0799x; 1.0713x over previous
"""Trainium2 Bass kernel for nn_AudioModel segment_reduce.

Reference computation (per batch row b):
  - frames t < audio_lengths[b] are valid
  - consecutive runs of equal phoneme_ids form segments
  - feat[b] = mean over segments of (mean over frames in segment of h[b,t,:])
  - logit[b] = feat[b] @ W.T + bias

Algebraic collapse: feat[b] = sum_t w[b,t] * h[b,t,:] with
  w[b,t] = valid[b,t] / (run_len(b, run_of(t)) * n_runs[b])
so  logit[b] = sum_t w[b,t] * (h[b,t,:] . W) + bias.

The per-frame weights w depend only on the tiny phoneme_ids/audio_lengths
tensors and are computed on host. The 588MB hidden_states tensor is streamed
through the device exactly once (memory-bound): per batch row, the T
contraction runs on DVE as a chain of fused scalar_tensor_tensor ops
(acc += w_chunk * h_chunk, per-partition scalar = per-frame weight), the
remaining chunks contract on the PE as fp32 matmuls accumulating in PSUM
(the DVE accumulator folds into the same PSUM group via a ones-matmul), and
one fused DVE op applies the classifier dot. Full fp32 throughout. Pure data
parallel: 16 batch rows per NeuronCore; each row's loads are split into
~1.1MB 3-chunk pieces across the two HWDGE rings (sync/scalar feed DVE/PE
respectively, alternating per row) with the ragged 88-frame tail on SWDGE,
so the HBM stream stays dense end-to-end and both pipeline edges are short.
"""

import numpy as np

B, T, C = 128, 1496, 768
NCORES = 8
RPC = B // NCORES          # batch rows per core
NCHUNK = 12                # t-chunks of 128 frames
NTAIL = T - 11 * 128       # 88 frames in the last chunk
DVE_CHUNKS = 6             # chunks 0..5 + tail on DVE, 6..10 on PE

_CACHE = {}


def _frame_weights(phoneme_ids, audio_lengths):
    """w[b,t] = valid / (run_length(run containing t) * n_runs[b]); 0 if invalid."""
    pid = np.asarray(phoneme_ids)
    L = np.asarray(audio_lengths).astype(np.int64)
    t = np.arange(T)
    valid = t[None, :] < L[:, None]                               # (B, T)
    change = pid[:, 1:] != pid[:, :-1]
    boundary = np.concatenate([np.ones((B, 1), bool), change], axis=1) & valid
    seg = np.cumsum(boundary, axis=1) - 1
    np.maximum(seg, 0, out=seg)                                   # (B, T)
    gid = (seg + np.arange(B, dtype=np.int64)[:, None] * T).ravel()
    cnt = np.bincount(gid, weights=valid.ravel().astype(np.float64), minlength=B * T)
    cnt_t = cnt[gid].reshape(B, T)                                # run length per frame
    n_runs = boundary.sum(axis=1).astype(np.float64)              # (B,)
    w = np.where(valid, 1.0 / (np.maximum(cnt_t, 1.0) * n_runs[:, None]), 0.0)
    return w.astype(np.float32)


def _weight_cols(w):
    """(B, 128, NCHUNK): [:, p, ci] = weight of frame ci*128+p (zeros pad the tail)."""
    wc = np.zeros((B, 128, NCHUNK), dtype=np.float32)
    wc[:, :, :11] = w[:, : 11 * 128].reshape(B, 11, 128).transpose(0, 2, 1)
    wc[:, :NTAIL, 11] = w[:, 11 * 128 :]
    return wc


def _build_program():
    import concourse.bacc as bacc
    import concourse.tile as tile
    from concourse import mybir

    f32 = mybir.dt.float32

    nc = bacc.Bacc("TRN2", target_bir_lowering=False, debug=False)
    h = nc.dram_tensor("h", [RPC, T, C], f32, kind="ExternalInput").ap()
    wt = nc.dram_tensor("wt", [128, RPC * NCHUNK], f32, kind="ExternalInput").ap()
    wv = nc.dram_tensor("wv", [1, C], f32, kind="ExternalInput").ap()
    out = nc.dram_tensor("out", [RPC, 1], f32, kind="ExternalOutput").ap()

    with tile.TileContext(nc) as tc:
        with (
            tc.tile_pool(name="hp", bufs=4) as hp,
            tc.tile_pool(name="const", bufs=1) as cp,
            tc.tile_pool(name="ap_", bufs=4) as apool,
            tc.tile_pool(name="ps", bufs=4, space="PSUM") as pp,
            tc.tile_pool(name="fin", bufs=3) as fp,
        ):
            wtile = cp.tile([128, RPC * NCHUNK], f32)
            nc.gpsimd.dma_start(wtile[:], wt)
            wvt = cp.tile([1, C], f32)
            nc.gpsimd.dma_start(wvt[:], wv)
            logits = cp.tile([1, RPC], f32)
            ones = cp.tile([128, 1], f32)
            nc.vector.memset(ones[:], 1.0)

            NA = DVE_CHUNKS            # chunks 0..NA-1 -> DVE (tile A, ring 0)
            NB = 11 - NA               # chunks NA..10  -> PE  (tile B, ring 1)
            for r in range(RPC):
                # Per-row loads are split so each engine's data arrives on its
                # own ring and compute starts after a half-row, not a full row:
                #   A: chunks 0..NA-1  -> DVE accumulator chain
                #   B: chunks NA..10   -> PE fp32 matmul group
                #   T: ragged 88-frame tail (SWDGE) -> one more DVE chain op
                ra, rb = (nc.sync, nc.scalar) if r % 2 == 0 else (nc.scalar, nc.sync)
                hA = hp.tile([128, NA * C], f32, tag="hA")
                hA3 = hA.rearrange("p (ci c) -> p ci c", c=C)
                for lo, hi in ((0, 3), (3, NA)):
                    ra.dma_start(
                        hA3[:, lo:hi, :],
                        h[r, lo * 128 : hi * 128, :].rearrange(
                            "(ci p) c -> p ci c", p=128
                        ),
                    )
                hB = hp.tile([128, NB * C], f32, tag="hB")
                hB3 = hB.rearrange("p (ci c) -> p ci c", c=C)
                for lo, hi in ((0, 3), (3, NB)):
                    rb.dma_start(
                        hB3[:, lo:hi, :],
                        h[r, (NA + lo) * 128 : (NA + hi) * 128, :].rearrange(
                            "(ci p) c -> p ci c", p=128
                        ),
                    )
                hT = hp.tile([128, C], f32, tag="hT")
                ra.dma_start(hT[:NTAIL, :], h[r, 11 * 128 :, :])

                col = lambda ci: wtile[:, r * NCHUNK + ci : r * NCHUNK + ci + 1]
                acc = apool.tile([128, C], f32, tag="acc")
                nc.vector.tensor_scalar_mul(acc[:], hA[:, :C], col(0))
                for ci in range(1, NA):
                    nc.vector.scalar_tensor_tensor(
                        out=acc[:],
                        in0=hA[:, ci * C : (ci + 1) * C],
                        scalar=col(ci),
                        in1=acc[:],
                        op0=mybir.AluOpType.mult,
                        op1=mybir.AluOpType.add,
                    )
                nc.vector.scalar_tensor_tensor(
                    out=acc[:NTAIL],
                    in0=hT[:NTAIL, :],
                    scalar=col(11)[:NTAIL],
                    in1=acc[:NTAIL],
                    op0=mybir.AluOpType.mult,
                    op1=mybir.AluOpType.add,
                )

                p = pp.tile([1, C], f32, tag="p")
                for k, ci in enumerate(range(NA, 11)):
                    lw = col(ci)
                    nc.tensor.matmul(
                        p[:, :512], lw, hB3[:, k, :512], start=(k == 0), stop=False
                    )
                    nc.tensor.matmul(
                        p[:, 512:], lw, hB3[:, k, 512:], start=(k == 0), stop=False
                    )
                nc.tensor.matmul(p[:, :512], ones[:], acc[:, :512], start=False, stop=True)
                nc.tensor.matmul(p[:, 512:], ones[:], acc[:, 512:], start=False, stop=True)
                sc = fp.tile([1, C], f32, tag="sc")
                nc.vector.scalar_tensor_tensor(
                    out=sc[:],
                    in0=p[:],
                    scalar=1.0,
                    in1=wvt[:],
                    op0=mybir.AluOpType.mult,
                    op1=mybir.AluOpType.mult,
                    accum_out=logits[:, r : r + 1],
                )

            nc.sync.dma_start(out.rearrange("r o -> o r"), logits[:])

    nc.compile()
    return nc


def _get_program():
    if "nc" not in _CACHE:
        _CACHE["nc"] = _build_program()
    return _CACHE["nc"]


def _run(inputs, trace=False):
    from concourse.bass_utils import run_bass_kernel_spmd

    hidden = np.ascontiguousarray(np.asarray(inputs["hidden_states"], dtype=np.float32))
    W = np.ascontiguousarray(np.asarray(inputs["W"], dtype=np.float32))
    bias = np.asarray(inputs["b"], dtype=np.float32)
    w = _frame_weights(inputs["phoneme_ids"], inputs["audio_lengths"])
    wc = _weight_cols(w)                                          # (B, 128, NCHUNK)

    in_maps = []
    for i in range(NCORES):
        r0 = i * RPC
        wt_core = np.ascontiguousarray(
            wc[r0 : r0 + RPC].transpose(1, 0, 2).reshape(128, RPC * NCHUNK)
        )
        in_maps.append(
            {
                "h": hidden[r0 : r0 + RPC],
                "wt": wt_core,
                "wv": W.reshape(1, C),
            }
        )

    nc = _get_program()
    res = run_bass_kernel_spmd(nc, in_maps, list(range(NCORES)), trace=trace)
    logit = np.concatenate([res.results[i]["out"] for i in range(NCORES)], axis=0)
    logit = logit + bias[None, :]
    return logit.astype(np.float32), res


def kernel(**inputs):
    return _run(inputs, trace=False)[0]
